# revision 5
# baseline (speedup 1.0000x reference)
"""Causal multi-head self-attention with RoPE on 8 Trainium2 NeuronCores. v3

Sharding: data parallel over batch (2) x tensor parallel over heads (4 groups
of 4 heads).  Core c handles batch b = c // 4, head group hg = c % 4.

v3 structure (j-major, pair-phased):
  - Heads are processed as 2 pairs (even head on partitions 0-63, odd head on
    64-127).  The pair's score matmuls run CONCURRENTLY on the two row-tiles
    of the PE array (tile_position (0,0)/(64,0)) -> 2x score throughput.
  - Pair phase: for j in 0..15: scores+exp for both heads of the pair;
    P^T V chains trail at each diagonal (chunk) boundary.  Pair 1's phase
    also carries the output projection per chunk; output DMA streams.
  - Q/K/V projections + RoPE for the inactive pair are interleaved as PE
    filler work so the Exp pipeline (the ACT engine, ~86us) never starves.
  - Softmax denominator: ones column in V_aug (M=65 matmuls); the reciprocal
    broadcast is a K=1 fp16 ones-matmul on the PE + 64-lane DVE reciprocal
    (no gpsimd custom ops).
  - Inputs restaged host-side; input DMA split across the SP + ACT HW-DGE
    queues and the gpsimd SW-DGE queue so the first matmul starts ~8us in.
"""

import numpy as np

import concourse.bass as bass
import concourse.mybir as mybir
import concourse.tile as tile
from concourse import bacc
from concourse.bass_utils import run_bass_kernel_spmd

F32 = mybir.dt.float32
F16 = mybir.dt.float16

B, S, D, H, DH = 2, 2048, 1024, 16, 64
ROPE_THETA = 10000.0
NCORE = 8
HPG = 4          # heads per group (per core)
P = 128
NKT = S // P     # 16 k-tiles
NQC = S // 512   # 4 query chunks

# per-head exp slab: k-tile j's columns live at [_OFF[j], _OFF[j]+S-128j)
_W = [S - P * j for j in range(NKT)]
_OFF = np.concatenate([[0], np.cumsum(_W)]).astype(int)
EXP_TOT = int(_OFF[-1])  # 17408 fp16 cols -> 34KB/partition per head


def build_program():
    nc = bacc.Bacc(
        "TRN2", target_bir_lowering=False, debug=False, num_devices=NCORE
    )

    # all inputs are host-restaged so each SBUF partition's data is one
    # contiguous DRAM run (big DMA descriptors -> full queue bandwidth)
    xts = nc.dram_tensor("xts", [NQC, P, 8, 512], F16, kind="ExternalInput")
    wq2 = nc.dram_tensor("wq2", [2, P, 8, P], F16, kind="ExternalInput")
    wk2 = nc.dram_tensor("wk2", [2, P, 8, P], F16, kind="ExternalInput")
    wvT = nc.dram_tensor("wvT", [P, 8, 256], F16, kind="ExternalInput")
    woT = nc.dram_tensor("woT", [P, 2, D], F16, kind="ExternalInput")
    cosT = nc.dram_tensor("cosT", [P, S], F16, kind="ExternalInput")
    sinT = nc.dram_tensor("sinT", [P, S], F16, kind="ExternalInput")
    ST = nc.dram_tensor("ST", [P, P], F16, kind="ExternalInput")
    trimask = nc.dram_tensor("trimask", [P, P], F16, kind="ExternalInput")

    outT = nc.dram_tensor("outT", [D, S], F16, kind="ExternalOutput")

    with tile.TileContext(nc) as tc:
        with (
            tc.tile_pool(name="big", bufs=1) as big,
            tc.tile_pool(name="tmp", bufs=3) as tmp,
            tc.tile_pool(name="psum", bufs=1, space="PSUM") as psum,
            tc.tile_pool(name="outp", bufs=4) as outp,
        ):
            xt_sb = big.tile([P, 8, S], F16, tag="xt")
            wq_sb = big.tile([P, 2, 8, P], F16, tag="wq")
            wk_sb = big.tile([P, 2, 8, P], F16, tag="wk")
            wv_sb = big.tile([P, 8, 256], F16, tag="wv")
            wo_sb = big.tile([P, 2, D], F16, tag="wo")
            cos_sb = big.tile([P, S], F16, tag="cos")
            sin_sb = big.tile([P, S], F16, tag="sin")
            st_sb = big.tile([P, P], F16, tag="st")
            tri_sb = big.tile([P, P], F16, tag="tri")
            ones_sb = big.tile([P, 512], F16, tag="ones")
            qrot = big.tile([P, 2, S], F16, tag="qrot")
            krot = big.tile([P, 2, S], F16, tag="krot")
            v_sb = big.tile([P, NKT, HPG, DH + 1], F16, tag="v")
            at_sb = big.tile([P, 2, S], F16, tag="at")
            # exp slabs for the ACTIVE pair: [even head | odd head]
            ep_sb = big.tile([P, 2, EXP_TOT], F16, tag="ep")

            # ---- input DMA across three queues ----
            # sync (SP hwdge): the big x stream
            for sc in range(2):
                nc.sync.dma_start(
                    out=xt_sb[:, 0:4, 512 * sc:512 * (sc + 1)],
                    in_=xts[sc, :, 0:4],
                )
                nc.gpsimd.dma_start(
                    out=xt_sb[:, 4:8, 512 * sc:512 * (sc + 1)],
                    in_=xts[sc, :, 4:8],
                )
            for sc in range(2, NQC):
                nc.sync.dma_start(
                    out=xt_sb[:, :, 512 * sc:512 * (sc + 1)],
                    in_=xts[sc],
                )
            # scalar (ACT hwdge): weights + rope constants (done before exps)
            nc.scalar.dma_start(out=wq_sb[:, 0], in_=wq2[0])
            nc.scalar.dma_start(out=wk_sb[:, 0], in_=wk2[0])
            nc.scalar.dma_start(out=sin_sb[:], in_=sinT[:, :])
            nc.scalar.dma_start(out=cos_sb[:], in_=cosT[:, :])
            nc.scalar.dma_start(out=st_sb[:], in_=ST[:, :])
            nc.scalar.dma_start(out=wq_sb[:, 1], in_=wq2[1])
            nc.scalar.dma_start(out=wk_sb[:, 1], in_=wk2[1])
            nc.scalar.dma_start(out=tri_sb[:], in_=trimask[:, :])
            # gpsimd (swdge): V and O weights
            nc.gpsimd.dma_start(out=wv_sb[:], in_=wvT[:])
            nc.gpsimd.dma_start(out=wo_sb[:], in_=woT[:])

            nc.vector.memset(ones_sb[:], 1.0)
            nc.vector.memset(v_sb[:, :, :, DH:DH + 1], 1.0)

            # warm the PE clock (HAM) with dummy matmuls while DMA streams in
            wsp = psum.tile([P, 512], F32, tag="sh", bufs=1, name="warm")
            for _ in range(14):
                nc.tensor.matmul(
                    wsp[0:DH, :],
                    ones_sb[0:1, 0:DH],
                    ones_sb[0:1, :],
                    start=True, stop=True,
                    tile_position=(0, 0),
                )

            # ---------------- building blocks -----------------
            def qk_pair(w_sb, rot, mt, scp):
                """Two 512-token slices of the Q or K projection + RoPE.
                dt-outer loop so each weight tile is loaded once for both
                chunk streams (halves the LDWEIGHTS count)."""
                ssls = [bass.ts(sc, 512) for sc in scp]
                pps = [
                    psum.tile([P, 512], F32, tag="fill", bufs=2, name="pp")
                    for _ in scp
                ]
                for dt in range(8):
                    for i, ssl in enumerate(ssls):
                        nc.tensor.matmul(
                            pps[i][:],
                            w_sb[:, mt, dt, :],
                            xt_sb[:, dt, ssl],
                            start=(dt == 0),
                            stop=(dt == 7),
                        )
                for i, ssl in enumerate(ssls):
                    pp = pps[i]
                    t_s = tmp.tile([P, 512], F16, tag="ts")
                    nc.vector.tensor_tensor(
                        out=t_s[:], in0=pp[:], in1=sin_sb[:, ssl],
                        op=mybir.AluOpType.mult,
                    )
                    sh = psum.tile([P, 512], F32, tag="sh", bufs=1, name="sh")
                    nc.tensor.matmul(
                        sh[:], st_sb[:], t_s[:], start=True, stop=True
                    )
                    nc.vector.tensor_tensor(
                        out=rot[:, mt, ssl], in0=pp[:], in1=cos_sb[:, ssl],
                        op=mybir.AluOpType.mult,
                    )
                    nc.vector.tensor_tensor(
                        out=rot[:, mt, ssl], in0=rot[:, mt, ssl], in1=sh[:],
                        op=mybir.AluOpType.add,
                    )

            def v_group(st):
                """V projection for one 128-token tile (all 4 local heads)."""
                vp = psum.tile([P, 512], F32, tag="fill", bufs=2, name="vp")
                for dt in range(8):
                    nc.tensor.matmul(
                        vp[:, 0:256],
                        xt_sb[:, dt, P * st:P * (st + 1)],
                        wv_sb[:, dt, :],
                        start=(dt == 0),
                        stop=(dt == 7),
                    )
                nc.vector.tensor_copy(
                    out=v_sb[:, st, :, 0:DH],
                    in_=vp[:, 0:256].rearrange("p (h d) -> p h d", h=HPG),
                )

            def scores_piece(pair, j, qa, qb):
                """Scores + exp for k-tile j, both heads of the pair, for
                absolute query columns [qa, qb) (each <=1024 wide).  Applies
                the diagonal causal mask when the piece starts at q=128j."""
                off = int(_OFF[j])
                q0 = P * j
                w = qb - qa
                tags = ("sE", "sO")
                sps = [
                    psum.tile([P, 1024], F32, tag=tags[h2], bufs=1,
                              name=tags[h2])
                    for h2 in range(2)
                ]
                # interleave E/O so the two PE row-tiles stream concurrently
                for mpos in range(0, w, 512):
                    mw = min(512, w - mpos)
                    for h2 in range(2):
                        base = 64 * h2
                        nc.tensor.matmul(
                            sps[h2][:, mpos:mpos + mw],
                            krot[base:base + 64, pair, q0:q0 + P],
                            qrot[base:base + 64, pair,
                                 qa + mpos:qa + mpos + mw],
                            start=True, stop=True,
                            tile_position=(base, 0),
                        )
                for h2 in range(2):
                    nc.scalar.activation(
                        out=ep_sb[:, h2, off + qa - q0:off + qb - q0],
                        in_=sps[h2][:, 0:w],
                        func=mybir.ActivationFunctionType.Exp,
                        scale=0.125,
                    )
                if qa == q0:
                    for h2 in range(2):
                        nc.vector.tensor_tensor(
                            out=ep_sb[:, h2, off:off + P],
                            in0=ep_sb[:, h2, off:off + P],
                            in1=tri_sb[:],
                            op=mybir.AluOpType.mult,
                        )

            def pv_head(pair, h2, c):
                """P^T V for one head and one query chunk + normalize."""
                h = 2 * pair + h2
                last = 4 * c + 3
                pv = psum.tile([P, 512], F32, tag="pv", bufs=1, name="pv")
                for j in range(last + 1):
                    off = int(_OFF[j])
                    if j // 4 == c:
                        r = j % 4
                        n = 512 - P * r
                        nc.tensor.matmul(
                            pv[0:DH + 1, P * r:512],
                            v_sb[:, j, h, :],
                            ep_sb[:, h2, off:off + n],
                            start=(j == 0), stop=(j == last),
                        )
                    else:
                        st_col = off + 512 * c - P * j
                        nc.tensor.matmul(
                            pv[0:DH + 1, :],
                            v_sb[:, j, h, :],
                            ep_sb[:, h2, st_col:st_col + 512],
                            start=(j == 0), stop=(j == last),
                        )
                pvb = tmp.tile([P, 512], F16, tag="pvb", bufs=2)
                nc.vector.tensor_copy(out=pvb[0:DH + 1, :], in_=pv[0:DH + 1, :])
                bc = psum.tile([P, 512], F32, tag="sh", bufs=1, name="bc")
                nc.tensor.matmul(
                    bc[0:DH, :],
                    ones_sb[DH:DH + 1, 0:DH],
                    pvb[DH:DH + 1, :],
                    start=True, stop=True,
                    tile_position=(64, 0),
                )
                rec = tmp.tile([P, 512], F32, tag="rec", bufs=2)
                nc.vector.reciprocal_approx_fast(
                    out=rec[0:DH, :], in_=bc[0:DH, :]
                )
                nc.vector.tensor_tensor(
                    out=at_sb[64 * h2:64 * h2 + 64, pair, bass.ts(c, 512)],
                    in0=pvb[0:DH, :], in1=rec[0:DH, :],
                    op=mybir.AluOpType.mult,
                )

            def outproj_piece(c, ot):
                """One 128-row slab of the output projection for chunk c."""
                ssl = bass.ts(c, 512)
                osl = bass.ts(ot, P)
                po = psum.tile([P, 512], F32, tag="fill", bufs=2, name="po")
                for ct in range(2):
                    nc.tensor.matmul(
                        po[:],
                        wo_sb[:, ct, osl],
                        at_sb[:, ct, ssl],
                        start=(ct == 0), stop=(ct == 1),
                    )
                ob = outp.tile([P, 512], F16, tag="ob")
                nc.vector.tensor_copy(out=ob[:], in_=po[:])
                if ot % 2 == 0:
                    nc.sync.dma_start(out=outT[osl, ssl], in_=ob[:])
                else:
                    nc.gpsimd.dma_start(out=outT[osl, ssl], in_=ob[:])

            # ---------------- schedule -----------------
            # Piece order within a pair phase: A-half pieces [128j, 1024) for
            # j=0..7, then the [1024, 2048) halves and the j>=8 tiles, with
            # pv chains at each chunk boundary and fillers woven between
            # pieces.  pv(c) is legal once every k-tile j<=4c+3 has its
            # chunk-c columns exp'd.
            # Event kinds: ("s", j, qa, qb) score piece; ("pv", h2, c);
            # ("qk", w, rot, mt, sc); ("v", st); ("op", c) queue outproj.
            def th0_events():
                q, k = "q", "k"
                ev = []
                ev += [("s", 0, 0, 1024), ("v", 0),
                       ("s", 1, 128, 1024), ("v", 1),
                       ("s", 2, 256, 1024), ("v", 2),
                       ("s", 3, 384, 1024), ("v", 3),
                       ("pv", 0, 0), ("pv", 1, 0), ("v", 4),
                       ("s", 4, 512, 1024), ("v", 5),
                       ("s", 5, 640, 1024), ("v", 6),
                       ("s", 6, 768, 1024), ("v", 7),
                       ("s", 7, 896, 1024), ("qk", q, 0, (2, 3)),
                       ("pv", 0, 1), ("pv", 1, 1),
                       ("qk", k, 0, (2, 3)),
                       ("s", 0, 1024, 2048),
                       ("s", 1, 1024, 2048), ("qk", q, 1, (0, 1)),
                       ("s", 2, 1024, 2048),
                       ("s", 3, 1024, 2048), ("qk", k, 1, (0, 1)),
                       ("s", 8, 1024, 2048),
                       ("s", 9, 1152, 2048), ("v", 8),
                       ("s", 10, 1280, 2048), ("v", 9),
                       ("s", 11, 1408, 2048), ("v", 10),
                       ("s", 4, 1024, 2048), ("v", 11),
                       ("s", 5, 1024, 2048), ("qk", q, 1, (2, 3)),
                       ("s", 6, 1024, 2048),
                       ("s", 7, 1024, 2048), ("qk", k, 1, (2, 3)),
                       ("pv", 0, 2), ("pv", 1, 2), ("v", 12),
                       ("s", 12, 1536, 2048), ("v", 13),
                       ("s", 13, 1664, 2048), ("v", 14),
                       ("s", 14, 1792, 2048), ("v", 15),
                       ("s", 15, 1920, 2048),
                       ("pv", 0, 3), ("pv", 1, 3)]
                return ev

            def th1_events():
                ev = []
                ev += [("s", 0, 0, 1024),
                       ("s", 1, 128, 1024),
                       ("s", 2, 256, 1024),
                       ("s", 3, 384, 1024),
                       ("pv", 0, 0), ("pv", 1, 0), ("op", 0),
                       ("s", 4, 512, 1024),
                       ("s", 5, 640, 1024),
                       ("s", 6, 768, 1024),
                       ("s", 7, 896, 1024),
                       ("pv", 0, 1), ("pv", 1, 1), ("op", 1),
                       ("s", 0, 1024, 2048),
                       ("s", 1, 1024, 2048),
                       ("s", 2, 1024, 2048),
                       ("s", 3, 1024, 2048),
                       ("s", 8, 1024, 2048),
                       ("s", 9, 1152, 2048),
                       ("s", 10, 1280, 2048),
                       ("s", 11, 1408, 2048),
                       ("s", 4, 1024, 2048),
                       ("s", 5, 1024, 2048),
                       ("s", 6, 1024, 2048),
                       ("s", 7, 1024, 2048),
                       ("pv", 0, 2), ("pv", 1, 2), ("op", 2),
                       ("s", 12, 1536, 2048),
                       ("s", 13, 1664, 2048),
                       ("s", 14, 1792, 2048),
                       ("s", 15, 1920, 2048),
                       ("pv", 0, 3), ("pv", 1, 3), ("op", 3)]
                return ev

            def run_phase(pair, events):
                ready_out = []

                def weave(n):
                    k = 0
                    while k < n and ready_out:
                        ready_out.pop(0)()
                        k += 1

                for ev in events:
                    if ev[0] == "s":
                        _, j, qa, qb = ev
                        pos = qa
                        while pos < qb:
                            w = min(1024, qb - pos)
                            scores_piece(pair, j, pos, pos + w)
                            pos += w
                            weave(2)
                    elif ev[0] == "pv":
                        _, h2, c = ev
                        pv_head(pair, h2, c)
                        weave(1)
                    elif ev[0] == "op":
                        ready_out.extend(
                            [lambda c=ev[1], ot=ot: outproj_piece(c, ot)
                             for ot in range(8)]
                        )
                        weave(2)
                    elif ev[0] == "qk":
                        _, w, mt, scp = ev
                        if w == "q":
                            qk_pair(wq_sb, qrot, mt, scp)
                        else:
                            qk_pair(wk_sb, krot, mt, scp)
                    elif ev[0] == "v":
                        v_group(ev[1])
                while ready_out:
                    ready_out.pop(0)()

            # Preload: just enough Q/K for pair-0's first A pieces.
            qk_pair(wq_sb, qrot, 0, (0, 1))
            qk_pair(wk_sb, krot, 0, (0, 1))

            run_phase(0, th0_events())
            run_phase(1, th1_events())

    nc.compile()
    return nc


_PROGRAM = None


def _get_program():
    global _PROGRAM
    if _PROGRAM is None:
        _PROGRAM = build_program()
    return _PROGRAM


def _host_consts(token_positions):
    pos = np.asarray(token_positions, dtype=np.float32)
    inv = (
        ROPE_THETA ** (-np.arange(0, DH, 2, dtype=np.float32) / DH)
    ).astype(np.float32)
    ang = pos[:, None] * inv[None, :]  # [S, 32]
    cos, sin = np.cos(ang), np.sin(ang)
    rows = (np.arange(P) % DH) // 2
    cosT = np.ascontiguousarray(cos.T[rows]).astype(np.float16)
    sinT = np.ascontiguousarray(sin.T[rows]).astype(np.float16)
    Smat = np.zeros((P, P), dtype=np.float32)
    idx = np.arange(0, P, 2)
    Smat[idx, idx + 1] = -1.0
    Smat[idx + 1, idx] = 1.0
    ST = np.ascontiguousarray(Smat.T).astype(np.float16)
    tri = (np.arange(P)[None, :] >= np.arange(P)[:, None]).astype(np.float16)
    return cosT, sinT, ST, tri


def _make_in_maps(x, W_q, W_k, W_v, W_o, token_positions):
    cosT, sinT, ST, tri = _host_consts(token_positions)
    x = np.asarray(x, dtype=np.float32)
    maps = []
    for core in range(NCORE):
        b, hg = core // 4, core % 4
        hsl = slice(256 * hg, 256 * (hg + 1))
        xT = x[b].T.astype(np.float16)                      # [1024, 2048]
        # [sc, p, dt, m]: per (sc, partition) one contiguous 8KB run
        xts = np.ascontiguousarray(
            xT.reshape(8, P, NQC, 512).transpose(2, 1, 0, 3)
        )                                                   # [4, 128, 8, 512]
        wqT = np.asarray(W_q, np.float32)[hsl].T            # [1024, 256]
        wkT = np.asarray(W_k, np.float32)[hsl].T
        # [mt, p, dt, m]
        wq2 = np.ascontiguousarray(
            wqT.reshape(8, P, 2, P).transpose(2, 1, 0, 3)
        ).astype(np.float16)                                # [2, 128, 8, 128]
        wk2 = np.ascontiguousarray(
            wkT.reshape(8, P, 2, P).transpose(2, 1, 0, 3)
        ).astype(np.float16)
        wvs = np.asarray(W_v, np.float32)[hsl].T            # [1024, 256]
        wvs = np.ascontiguousarray(
            wvs.reshape(8, P, 256).transpose(1, 0, 2)
        ).astype(np.float16)                                # [128, 8, 256]
        wo_p = np.asarray(W_o, dtype=np.float32)[:, hsl].T  # [256, 1024]
        wo_p = np.ascontiguousarray(
            wo_p.reshape(2, P, D).transpose(1, 0, 2)
        ).astype(np.float16)                                # [128, 2, 1024]
        maps.append(
            {
                "xts": xts,
                "wq2": wq2,
                "wk2": wk2,
                "wvT": wvs,
                "woT": wo_p,
                "cosT": cosT,
                "sinT": sinT,
                "ST": ST,
                "trimask": tri,
            }
        )
    return maps


def _assemble(results):
    out = np.zeros((B, S, D), dtype=np.float32)
    for core in range(NCORE):
        b = core // 4
        out[b] += results[core]["outT"].astype(np.float32).T
    return out


def _run(in_maps, trace=False):
    nc = _get_program()
    tmpdir = None
    if trace:
        import tempfile

        tmpdir = tempfile.mkdtemp(prefix="ntff_", dir="/tmp")
    res = run_bass_kernel_spmd(
        nc, in_maps, list(range(NCORE)), trace=trace, tmpdir=tmpdir
    )
    return res


def kernel(x, W_q, W_k, W_v, W_o, token_positions):
    in_maps = _make_in_maps(x, W_q, W_k, W_v, W_o, token_positions)
    res = _run(in_maps)
    return _assemble(res.results)


def _install_profile_hook():
    """The agent image's antenv lacks axon_hooks; shim it so trace=True works."""
    import sys
    import types

    try:
        from antenv.axon_hooks import get_axon_ntff_profile_hook  # noqa: F401
        return
    except ImportError:
        pass
    import antenv
    from trn_agent_boot.trn_boot import _ntff_profile_via_ctypes

    mod = types.ModuleType("antenv.axon_hooks")
    _hook = {"h": None}
    mod.set_axon_ntff_profile_hook = lambda h: _hook.__setitem__("h", h)
    mod.get_axon_ntff_profile_hook = lambda: _hook["h"]
    sys.modules["antenv.axon_hooks"] = mod
    antenv.axon_hooks = mod
    mod.set_axon_ntff_profile_hook(
        _ntff_profile_via_ctypes("/opt/axon/libaxon_pjrt.so")
    )
    import concourse.bass_utils as bu

    bu.upload_artifacts = lambda d: f"file://{d}"


def kernel_traced(x, W_q, W_k, W_v, W_o, token_positions):
    """Returns (output, exec_time_ns, trace_path)."""
    _install_profile_hook()
    in_maps = _make_in_maps(x, W_q, W_k, W_v, W_o, token_positions)
    res = _run(in_maps, trace=True)
    trace_path = None
    if res.instructions_and_trace is not None:
        trace_path = res.instructions_and_trace[1]
    return _assemble(res.results), res.exec_time_ns, trace_path
